# revision 64
# baseline (speedup 1.0000x reference)
"""LSH similarity-matrix kernel for Trainium2 (8 NeuronCores, data-parallel over batch).

Math: reference computes, per (l, b):
    c1 = (query_embed @ r.T > 0),  c2 = (doc_embed @ r.T > 0)   in {0,1}
    ham = s1 + s2 - 2*c1@c2.T ;  sim = cos(pi/NB * ham), masked where tok==0.
With +-1 codes U = 2c-1 and S = U1 @ U2.T:  ham = (NB - S)/2, so
    sim = sin(pi/(2*NB) * S).
Masks fold into the embeddings: a zeroed embedding row projects to 0,
clamp(0) = 0 gives a zero code row, so S = 0 and sin(0) = 0 — exactly the
masked output. Masked doc tokens (half of them: tok in {0,1}) are gathered
away host-side entirely; output columns scatter back as zeros. Batches are
assigned to (core, slot) sorted by active-token count so every core runs an
identically-shaped program with minimal padding per slot.

Precision: projections run as single tf32 (float32r) matmuls at 1 cycle/row.
tf32's 11-bit mantissa flips ~1.5k of the 71M hash bits vs the fp32
reference (sim absmax ~9e-3, rel err ~1e-4) — far inside the tolerance.
Inputs are pre-rounded to tf32 host-side and DMA'd straight into float32r
tiles, so no on-device conversion copies are needed. The code dot runs as
fp8e4m3 DoubleRow matmuls (chunk pairs give K=256 per MM at 2 MACs/cell/
cycle); +-1/0 codes and their fp32 PSUM accumulation are exact.

r is pre-scaled by 2^66 host-side so the sign alternative
clamp(x, -1, 1) = max(min(x,1),-1) is exact (any |proj| > 2^-66 maps to
+-1). With single-pass projections the binding constraint becomes the
PSUM->SBUF sign drain (GPSIMD cannot read PSUM; fp8 overflows to inf on
this target, so no saturation tricks — only DVE tensor_scalar clamps and
ACT Sign activations can drain, at ~1 column/cycle each). Hence:
 - chunk signs are split DVE/ACT by a greedy balancer whose costs are
   charged at the point in the job stream where each op actually runs
   (TUNE holds sweep-picked knobs for the balancer bias/tie-breaks);
 - the two L-layer jobs of a slot share one epilogue: their code dots
   accumulate into one PSUM tile with job l's rows at partitions 64*l
   (via zero-padded stationary windows in U1 — the ISA rejects DoubleRow
   matmuls at dst base 64, so the placement comes from the stationary
   column index), so one Sin and one output DMA cover both jobs;
 - single-bank (pad<=512) slots drain chunk pairs in one fused op;
 - act-table loads are forced into the PE warm-up window by dummy
   activations, and dot pieces are interleaved between projection chunk
   groups so the sign engines never starve behind the PE's dot blocks.
Output is written fp16 (sin in [-1,1]; exact to 2^-12) and upcast on the
host. Single-tf32 projection puts the end-to-end rel err at ~7e-3
(theory for tf32 operands: ~9e-3), ~3x inside the 2e-2 gate.
"""
import os
import sys

sys.path.insert(0, "/opt/trn_rl_repo")

from contextlib import ExitStack

import numpy as np

import concourse.bass as bass
import concourse.mybir as mybir
import concourse.tile as tile
from concourse import bacc
from concourse.bass_utils import run_bass_kernel_spmd

L, BAT, A, BDOC, D, NB = 2, 32, 64, 1024, 128, 1024
CORES = 8
BPC = BAT // CORES          # batch slots per core
CH = NB // 128              # 8 bit-chunks
SCALE = float(2.0 ** 66)
PI = float(np.pi)

F32 = mybir.dt.float32
F32R = mybir.dt.float32r
BF16 = mybir.dt.bfloat16
FP16 = mybir.dt.float16
FP8 = mybir.dt.float8e4
Alu = mybir.AluOpType
Act = mybir.ActivationFunctionType

NWARM = 7                   # PE ramp dummy matmuls (512 cols each)

# schedule-tuning knobs (values picked by the offline TimelineSim sweep)
TUNE = {
    "nwarm": 6,
    "dve_bias": 550.0,      # initial ns handicap on DVE in the sign balancer
    "interleave": True,     # emit dot pieces between projection chunk groups
    "job_order": "desc",    # slot order: desc | asc | valley
    "jitter": 70,           # balancer tie-break jitter seed (0 = off)
    "ehp_bufs": 4,
    "u2p_bufs": 4,
    "act_tail_dma": False,  # issue the final output DMA from the ACT queue
}

_BUILD_CACHE: dict = {}


def _col_splits(n):
    """Split [0, n) into equal-width pieces of <=512 columns (>=256 keeps
    float32r matmuls at full rate; a matmul may not cross a PSUM bank, so
    piece i is written at PSUM column 512*i). Equal widths mean one strided
    [p, npieces, w] access pattern covers all pieces, so sign/sin run as a
    single instruction per chunk. Returns (c0, c1, p0) per piece."""
    npieces = -(-n // 512)
    w = -(-(n // npieces) // 16) * 16
    while w * npieces < n:
        w += 16
    assert w * npieces >= n and w <= 512 and npieces <= 2
    return [(i * w, min((i + 1) * w, n), 512 * i) for i in range(npieces)]


def _slot_order_of(pads_c):
    desc = sorted(range(len(pads_c)), key=lambda s: -pads_c[s])
    if TUNE["job_order"] == "asc":
        return desc[::-1]
    if TUNE["job_order"] == "valley":
        # big slots in the middle, small first and last
        asc = desc[::-1]
        return [asc[0]] + desc[:-1]
    return desc


def _sign_plan(pads_c, qpad):
    """Assign each (job, chunk) doc sign to 'dve' or 'act' greedily by
    modelled engine cost (ns): DVE tensor_scalar = free*1.042 + 125,
    ACT activation = free*0.833 + 143. Costs are charged at the point in
    the job stream where the work actually runs (sin of job j-1 lands
    during job j; query-pair signs land around job 1; act-table loads are
    hoisted to the idle warm-up window) so the split is balanced in TIME,
    not just in total. Jobs run slots in descending-pad order, L-major."""
    QW = BPC * L * qpad
    order = _slot_order_of(pads_c)
    jobs = [(b, l) for b in order for l in range(L)]
    rng = None
    if TUNE["jitter"]:
        import random
        rng = random.Random(TUNE["jitter"])
    dve = float(TUNE["dve_bias"])
    act = 0.0
    plan = []
    for j, (b, _l) in enumerate(jobs):
        if j == 2:
            dve += 2 * (2 * QW * 1.042 + 125.0)
            act += 2 * (2 * QW * 0.833 + 185.0)  # 2 fused pairs per engine
        if j in (2, 4, 6) and j > 2:
            # fused L-pair Sin epilogues land during jobs 4/6 (first at 2)
            act += pads_c[jobs[j - 4][0]] * 0.833 + 212.0
        if j == 3:
            act += pads_c[jobs[0][0]] * 0.833 + 212.0
        row = []
        jit = (lambda: 1.0 + 0.25 * (rng.random() - 0.5)) if rng else (lambda: 1.0)
        if pads_c[b] <= 512:
            # np=1 slots drain chunk PAIRS in one fused op
            for _kk in range(CH // 2):
                cd = (2 * pads_c[b] * 1.042 + 125.0) * jit()
                ca = (2 * pads_c[b] * 0.833 + 185.0) * jit()
                if dve + cd <= act + ca:
                    dve += cd
                    row += ["dve", None]
                else:
                    act += ca
                    row += ["act", None]
        else:
            row = [None] * CH
            for g0, nk in ((0, 2), (2, 1), (3, 2), (5, 1), (6, 2)):
                free = nk * pads_c[b]
                cd = (free * 1.042 + 125.0) * jit()
                ca = (free * 0.833 + 185.0) * jit()
                if dve + cd <= act + ca:
                    dve += cd
                    row[g0] = "dve"
                else:
                    act += ca
                    row[g0] = "act"
        plan.append(row)
    return plan


def _build(pads_c: tuple, qpad: int = A, reps: int = 1):
    """Per-core SPMD program. pads_c[b]: compute width (mult of 16) of batch
    slot b. reps > 1 re-emits the whole body (timing instrumentation only)."""
    pads_c = tuple(int(p) for p in pads_c)
    pad_cmax = max(pads_c)
    slot_splits = [_col_splits(p) for p in pads_c]
    QW = BPC * L * qpad
    sign_plan = _sign_plan(pads_c, qpad)

    nc = bacc.Bacc("TRN2", target_bir_lowering=False, debug=False)

    QE = nc.dram_tensor("qe", [D, QW], F32R, kind="ExternalInput").ap()
    DE = nc.dram_tensor("de", [BPC, L, D, pad_cmax], F32R, kind="ExternalInput").ap()
    RT = nc.dram_tensor("rt", [D, NB], F32R, kind="ExternalInput").ap()
    OUT = nc.dram_tensor("out", [BPC, L * qpad, pad_cmax], FP16, kind="ExternalOutput").ap()

    def sign_to(eng, u, pp):
        if eng == "dve":
            nc.vector.tensor_scalar(u, pp, 1.0, -1.0, Alu.min, Alu.max)
        else:
            nc.scalar.activation(u, pp, Act.Sign)

    with tile.TileContext(nc) as tc, ExitStack() as ctx:
        const = ctx.enter_context(tc.tile_pool(name="const", bufs=1))
        ehp = ctx.enter_context(tc.tile_pool(name="ehp",
                                             bufs=int(TUNE["ehp_bufs"])))
        u2p = ctx.enter_context(tc.tile_pool(name="u2p",
                                             bufs=int(TUNE["u2p_bufs"])))
        outp = ctx.enter_context(tc.tile_pool(name="outp", bufs=2))
        # 8 PSUM banks: 2 x 2-bank rotating chunk tiles + 2 x 2-bank S tiles
        # (two S bufs so job j's dots never wait on job j-1's Sin drain)
        ps_p = ctx.enter_context(tc.tile_pool(name="ps_p", bufs=1, space="PSUM"))
        ps_s = ctx.enter_context(tc.tile_pool(name="ps_s", bufs=1, space="PSUM"))

        for _rep in range(reps):
            _rp = f"r{_rep}_"
            # ---- constants: rt arrives in pieces so the first projection
            # chunk unblocks as early as possible; everything lands directly
            # in float32r tiles (host pre-rounds to tf32) ----
            rhl = const.tile([D, NB], F32R, tag="rhl", name=f"{_rp}rhl")
            qh = const.tile([D, QW], F32R, tag="qh", name=f"{_rp}qh")
            # U1 block per (chunk k, slot b) is 192 cols: [q(l=0) | 64 zeros
            # | q(l=1)].  The dot's stationary for job (b,l) is the 128-col
            # window at l*64: [q_l0|zeros] or [zeros|q_l1], which places job
            # l's S rows at partitions l*64 of a SINGLE accumulation chain —
            # PE-free L-pair fusion of the epilogue (dst base stays 0; the
            # ISA rejects DoubleRow matmuls with dst base 64).
            UB = 3 * qpad                     # 192: U1 block width
            U1 = const.tile([D, CH * BPC * UB], FP8, tag="U1",
                            name=f"{_rp}U1")
            u1_pitch, u1_off = U1[:].ap[0][0], U1[:].offset
            # zero the hole strips once (Pool is otherwise idle)
            holes = bass.AP(U1.tensor, u1_off + qpad,
                            [[u1_pitch, D], [UB, CH * BPC], [1, qpad]])

            _slot_order = _slot_order_of(pads_c)
            jobs = [(b, l) for b in _slot_order for l in range(L)]
            st = [dict() for _ in jobs]

            def stage_a(j, split=False):
                b, l = jobs[j]
                pad = pads_c[b]
                eh = ehp.tile([D, pad_cmax], F32R, tag="eh",
                              name=f"{_rp}eh{j}")[:, 0:pad]
                if split:
                    # piece-granular DMAs: the first projection piece can
                    # start one transfer earlier (deps are range-tracked)
                    for c0, c1, _ in slot_splits[b]:
                        nc.sync.dma_start(out=eh[:, c0:c1],
                                          in_=DE[b, l, :, c0:c1])
                else:
                    nc.sync.dma_start(out=eh, in_=DE[b, l, :, 0:pad])
                st[j]["eh"] = eh

            # DMA priority order: first doc job, first proj chunk weights,
            # the rest of the weights, second doc job, queries.
            nc.sync.dma_start(out=rhl[:, 0:128], in_=RT[:, 0:128])
            stage_a(0)
            nc.sync.dma_start(out=rhl[:, 128:512], in_=RT[:, 128:512])
            nc.sync.dma_start(out=rhl[:, 512:NB], in_=RT[:, 512:NB])
            stage_a(1)
            nc.sync.dma_start(out=qh, in_=QE)

            # PE pre-warm: dependency-free dummy matmuls pull the PE through
            # its cold/mid clock ramp while the first DMAs land, so the real
            # projections run at 2.4 GHz
            warm = const.tile([D, 512], BF16, tag="warm", name=f"{_rp}warm")
            nc.gpsimd.memset(warm, 0.0)
            # dummy Sign + Sin on the idle ACT engine so both act-table
            # loads are hoisted into the warm-up window instead of stalling
            # the pipeline at their first real use
            wact = const.tile([D, 32], BF16, tag="wact", name=f"{_rp}wact")
            nc.scalar.activation(wact[:, 0:16], warm[:, 0:16], Act.Sign)
            nc.scalar.activation(wact[:, 16:32], warm[:, 16:32], Act.Sin)
            # one persistent 6-bank projection region, rotated manually in
            # thirds of 1024 f32: a sign op can then span two adjacent
            # thirds as a plain strided view, fusing two chunks' drains
            # (pool tiles are distinct tensors, so only a shared region
            # makes cross-chunk fusion expressible); WAR hazards on reused
            # thirds are range-tracked by the tile framework
            PP = ps_p.tile([D, 3072], F32, tag="pp", name=f"{_rp}PP")
            wps = PP[:, 0:512]
            for i in range(int(TUNE["nwarm"])):
                nc.tensor.matmul(wps, warm[:, 0:128], warm,
                                 start=True, stop=True)
            # zero U1's hole strips once the warm tile is set (Pool is
            # otherwise idle; emitting it here keeps the warm memset first
            # in Pool's queue so the PE ramp starts immediately)
            nc.gpsimd.memset(holes, 0.0)

            def stage_b(j, ks):
                b, l = jobs[j]
                pad = pads_c[b]
                splits = slot_splits[b]
                npieces = len(splits)
                w = splits[0][1] - splits[0][0]
                assert npieces * w == pad
                eh = st[j]["eh"]
                if "U2" not in st[j]:
                    st[j]["U2"] = u2p.tile([D, CH * pad_cmax], FP8, tag="U2",
                                           name=f"{_rp}U2{j}")
                U2 = st[j]["U2"]
                # chunk k lives in PP third THIRD[k]; groups of consecutive
                # chunks in adjacent thirds drain in ONE sign op
                if npieces == 1:
                    # np=1: a chunk pair fits one third (cols 0/512)
                    for k0 in [k for k in ks if k % 2 == 0]:
                        t0 = 1024 * ((k0 // 2) % 3)
                        for h in range(2):
                            k = k0 + h
                            nc.tensor.matmul(PP[:, t0 + 512 * h:t0 + 512 * h + pad],
                                             rhl[:, k * 128:(k + 1) * 128],
                                             eh, start=True, stop=True)
                        ppv = PP[:, t0:t0 + 1024].rearrange(
                            "p (n c) -> p n c", c=512)[:, 0:2, 0:pad]
                        u2v = U2[:, k0 * pad:(k0 + 2) * pad] \
                            .rearrange("p (n c) -> p n c", c=pad)
                        sign_to(sign_plan[j][k0], u2v, ppv)
                    return
                # np=2: fusion groups (0,1)(2)(3,4)(5)(6,7); fused groups sit
                # in thirds 0+1, singles in third 2
                GRP = {0: (0, 2, 0), 2: (2, 3, 2048), 3: (3, 5, 0),
                       5: (5, 6, 2048), 6: (6, 8, 0)}
                for g0 in sorted(GRP):
                    lo, hi, t0 = GRP[g0]
                    if lo not in ks:
                        continue
                    for i, k in enumerate(range(lo, hi)):
                        base = t0 + 1024 * i
                        for c0, c1, p0 in splits:
                            nc.tensor.matmul(PP[:, base + p0:base + p0 + c1 - c0],
                                             rhl[:, k * 128:(k + 1) * 128],
                                             eh[:, c0:c1], start=True, stop=True)
                    nk = hi - lo
                    ppv = PP[:, t0:t0 + 1024 * nk].rearrange(
                        "p (n c) -> p n c", c=512)[:, 0:2 * nk, 0:w]
                    u2v = U2[:, lo * pad:hi * pad] \
                        .rearrange("p (n c) -> p n c", c=w)
                    sign_to(sign_plan[j][g0], u2v, ppv)

            def query_proj():
                # chunk pairs share one PSUM tile (cols 0 and 512); one
                # 5-level-AP sign per pair scatters both chunks' [b, l, q]
                # into their holey U1 blocks, alternating DVE/ACT so U1
                # completes before the first dot
                for kk in range(CH // 2):
                    t0 = 1024 * (kk % 3)
                    for h in range(2):
                        k = 2 * kk + h
                        nc.tensor.matmul(PP[:, t0 + 512 * h:t0 + 512 * h + QW],
                                         rhl[:, k * 128:(k + 1) * 128], qh,
                                         start=True, stop=True)
                    u1v = bass.AP(U1.tensor, u1_off + 2 * kk * BPC * UB,
                                  [[u1_pitch, D], [BPC * UB, 2], [UB, BPC],
                                   [2 * qpad, L], [1, qpad]])
                    qpv = bass.AP(PP.tensor, PP[:].offset + t0,
                                  [[PP[:].ap[0][0], D], [512, 2],
                                   [L * qpad, BPC], [qpad, L], [1, qpad]])
                    sign_to("dve" if kk % 2 == 0 else "act", u1v, qpv)

            def stage_c_piece(p, pi):
                """Dot chain for piece pi of the fused L-pair p (jobs 2p,
                2p+1 — same slot b, l=0/1): one 8-matmul accumulation chain
                writes both jobs' S rows (job l at partitions [64l, 64l+64)
                via the zero-padded stationary windows)."""
                j0 = 2 * p
                b, _ = jobs[j0]
                pad = pads_c[b]
                if "S" not in st[j0]:
                    st[j0]["S"] = ps_s.tile([L * qpad, 1024], F32, tag="S",
                                            name=f"{_rp}S{p}")
                S = st[j0]["S"]
                CW = BPC * UB
                c0, c1, p0 = slot_splits[b][pi]
                for i, j in enumerate((j0, j0 + 1)):
                    _, l = jobs[j]
                    U2 = st[j]["U2"]
                    for jj in range(CH // 2):
                        lw = U1[:, 2 * jj * CW:(2 * jj + 2) * CW] \
                            .rearrange("p (o c) -> p o c", o=2) \
                            [:, :, b * UB + l * qpad:b * UB + (l + 2) * qpad]
                        rv = U2[:, 2 * jj * pad:(2 * jj + 2) * pad] \
                            .rearrange("p (o c) -> p o c", o=2)[:, :, c0:c1]
                        nc.tensor.matmul(
                            S[:, p0:p0 + c1 - c0], lw, rv,
                            start=(i == 0 and jj == 0),
                            stop=(i == 1 and jj == CH // 2 - 1),
                            perf_mode=mybir.MatmulPerfMode.DoubleRow,
                        )

            def stage_c_tail(p):
                """Sin + output DMA for fused pair p (after all its dot
                pieces): a single Sin over all 128 partitions and a single
                DMA (the L dim is contiguous in OUT)."""
                j0 = 2 * p
                b, _ = jobs[j0]
                pad = pads_c[b]
                splits = slot_splits[b]
                npieces = len(splits)
                w = splits[0][1] - splits[0][0]
                S = st[j0]["S"]
                sim = outp.tile([L * qpad, pad_cmax], FP16, tag="sim",
                                name=f"{_rp}sim{p}")[:, 0:pad]
                if npieces == 1:
                    nc.scalar.activation(sim, S[:, 0:pad], Act.Sin,
                                         scale=PI / (2.0 * NB))
                else:
                    sv = S[:].rearrange("p (n c) -> p n c",
                                        c=512)[:, 0:npieces, 0:w]
                    mv = sim.rearrange("p (n c) -> p n c", c=w)
                    nc.scalar.activation(mv, sv, Act.Sin, scale=PI / (2.0 * NB))
                if TUNE["act_tail_dma"] and p == len(jobs) // 2 - 1:
                    # same-engine issue skips the ACT->SP semaphore hop on
                    # the drain-critical final output
                    nc.scalar.dma_start(out=OUT[b, :, 0:pad], in_=sim)
                else:
                    nc.sync.dma_start(out=OUT[b, :, 0:pad], in_=sim)

            # deeper pipeline: the fused epilogue of pair p is emitted in
            # the middle of b(2p+2), so dots/Sin/output overlap the next
            # pair's projections and the Sin lands mid-stream on ACT
            # instead of stalling it
            n = len(jobs)
            stage_b(0, range(CH))
            stage_a(2)
            stage_b(1, range(CH))
            query_proj()
            stage_a(3)
            for j in range(2, n):
                ci = j // 2 - 1 if j % 2 == 0 else None   # fused pair idx
                cn = len(slot_splits[jobs[2 * ci][0]]) if ci is not None else 0
                # np=1 slots sign chunk PAIRS, so their groups stay aligned
                if TUNE["interleave"]:
                    g = ((0, 2), (2, 4), (4, CH)) \
                        if len(slot_splits[jobs[j][0]]) == 1 \
                        else ((0, 3), (3, 5), (5, CH))
                    stage_b(j, range(*g[0]))
                    if ci is not None:
                        stage_c_piece(ci, 0)
                    stage_b(j, range(*g[1]))
                    if ci is not None:
                        if cn > 1:
                            stage_c_piece(ci, 1)
                        stage_c_tail(ci)
                    if j + 2 < n:
                        stage_a(j + 2)
                    stage_b(j, range(*g[2]))
                else:
                    stage_b(j, range(0, 4))
                    if ci is not None:
                        stage_c_piece(ci, 0)
                        if cn > 1:
                            stage_c_piece(ci, 1)
                        stage_c_tail(ci)
                    if j + 2 < n:
                        stage_a(j + 2)
                    stage_b(j, range(4, CH))
            stage_c_piece(n // 2 - 1, 0)
            if len(slot_splits[jobs[n - 2][0]]) > 1:
                stage_c_piece(n // 2 - 1, 1)
            stage_c_tail(n // 2 - 1)

    nc.compile()
    return nc


def _tf32(x):
    """Round-to-nearest-even fp32 -> tf32 (11-bit mantissa), bit-matching
    the PE's fp32_to_fp32r conversion."""
    u = np.ascontiguousarray(x, np.float32).view(np.uint32).astype(np.uint64)
    u = (u + 0x07FF + ((u >> 12) & 1)) & 0xFFFFFFFFFFFFF000
    return (u & 0xFFFFFFFF).astype(np.uint32).view(np.float32)


def _stage_inputs(query_embed, doc_embed, query_tok, doc_tok, r):
    query_embed = np.ascontiguousarray(query_embed, dtype=np.float32)
    doc_embed = np.ascontiguousarray(doc_embed, dtype=np.float32)
    r = np.ascontiguousarray(r, dtype=np.float32)

    qmask = (np.asarray(query_tok) != 0)
    dmask = (np.asarray(doc_tok) != 0)

    # sort batches by active count; slot s takes ranks [s*CORES, (s+1)*CORES)
    # spread across the 8 cores, so per-slot padding is tight and identical
    # on every core (SPMD requires one shape per slot)
    counts = dmask.sum(axis=1).astype(int)
    order = np.argsort(counts, kind="stable")
    assign = np.empty((CORES, BPC), dtype=int)   # assign[c, b] = batch id
    for s in range(BPC):
        for c in range(CORES):
            assign[c, s] = order[s * CORES + c]
    def _pad(n):
        # mult of 16; slots that split across two PSUM banks need halves
        # that are themselves mult of 16, so round those to mult of 32
        p = max(64, -(-n // 16) * 16)
        if p > 512:
            p = -(-n // 32) * 32
        return min(BDOC, p)

    pads_c = tuple(_pad(int(counts[assign[:, s]].max())) for s in range(BPC))
    pad_cmax = max(pads_c)

    # all 64 query rows are kept (masked rows zeroed -> zero codes -> zero
    # output rows, matching the reference); 64 keeps the two jobs of an
    # L-pair at PE-legal PSUM base partitions 0 and 64
    qe_m = query_embed * qmask[None, :, :, None].astype(np.float32)
    qpad = A
    rt = np.ascontiguousarray(_tf32(r.T * SCALE))

    idxs = [np.flatnonzero(dmask[g]) for g in range(BAT)]
    in_maps = []
    for c in range(CORES):
        # embeddings staged pre-transposed [D, tokens], pre-rounded to tf32
        # (value-exact under the f32r DMA interpretation)
        qe_c = np.zeros((D, BPC * L * qpad), dtype=np.float32)
        de_c = np.zeros((BPC, L, D, pad_cmax), dtype=np.float32)
        for b in range(BPC):
            g = assign[c, b]
            for li in range(L):
                col = (b * L + li) * qpad
                qe_c[:, col:col + A] = qe_m[li, g].T
            idx = idxs[g]
            de_c[b, :, :, :len(idx)] = doc_embed[:, g, idx].transpose(0, 2, 1)
        in_maps.append({"qe": _tf32(qe_c), "de": _tf32(de_c), "rt": rt})

    return in_maps, assign, idxs, pads_c, qpad


def kernel(query_embed, doc_embed, query_tok, doc_tok, r):
    in_maps, assign, idxs, pads_c, qpad = _stage_inputs(
        query_embed, doc_embed, query_tok, doc_tok, r)

    key = (pads_c, qpad)
    if key not in _BUILD_CACHE:
        _BUILD_CACHE[key] = _build(pads_c, qpad)
    nc = _BUILD_CACHE[key]

    res = run_bass_kernel_spmd(nc, in_maps, core_ids=list(range(CORES)))

    out = np.zeros((BAT, L, A, BDOC), dtype=np.float32)
    for c in range(CORES):
        o_c = np.asarray(res.results[c]["out"]).astype(np.float32)
        o_c = o_c.reshape(BPC, L, A, -1)
        for b in range(BPC):
            g = assign[c, b]
            idx = idxs[g]
            for li in range(L):
                out[g, li][:, idx] = o_c[b, li, :, :len(idx)]
    return out


# revision 65
# speedup vs baseline: 2.1679x; 2.1679x over previous
"""LSH similarity-matrix kernel for Trainium2 (8 NeuronCores, data-parallel over batch).

Math: reference computes, per (l, b):
    c1 = (query_embed @ r.T > 0),  c2 = (doc_embed @ r.T > 0)   in {0,1}
    ham = s1 + s2 - 2*c1@c2.T ;  sim = cos(pi/NB * ham), masked where tok==0.
With +-1 codes U = 2c-1 and S = U1 @ U2.T:  ham = (NB - S)/2, so
    sim = sin(pi/(2*NB) * S).
Masks fold into the embeddings: a zeroed embedding row projects to 0,
clamp(0) = 0 gives a zero code row, so S = 0 and sin(0) = 0 — exactly the
masked output. Masked doc tokens (half of them: tok in {0,1}) are gathered
away host-side entirely; output columns scatter back as zeros. Batches are
assigned to (core, slot) sorted by active-token count so every core runs an
identically-shaped program with minimal padding per slot.

Precision: projections run as single tf32 (float32r) matmuls at 1 cycle/row.
tf32's 11-bit mantissa flips ~1.5k of the 71M hash bits vs the fp32
reference (sim absmax ~9e-3, rel err ~1e-4) — far inside the tolerance.
Inputs are pre-rounded to tf32 host-side and DMA'd straight into float32r
tiles, so no on-device conversion copies are needed. The code dot runs as
fp8e4m3 DoubleRow matmuls (chunk pairs give K=256 per MM at 2 MACs/cell/
cycle); +-1/0 codes and their fp32 PSUM accumulation are exact.

r is pre-scaled by 2^66 host-side so the sign alternative
clamp(x, -1, 1) = max(min(x,1),-1) is exact (any |proj| > 2^-66 maps to
+-1). With single-pass projections the binding constraint becomes the
PSUM->SBUF sign drain (GPSIMD cannot read PSUM; fp8 overflows to inf on
this target, so no saturation tricks — only DVE tensor_scalar clamps and
ACT Sign activations can drain, at ~1 column/cycle each). Hence:
 - chunk signs are split DVE/ACT by a greedy balancer whose costs are
   charged at the point in the job stream where each op actually runs
   (TUNE holds sweep-picked knobs for the balancer bias/tie-breaks);
 - the two L-layer jobs of a slot share one epilogue: their code dots
   accumulate into one PSUM tile with job l's rows at partitions 64*l
   (via zero-padded stationary windows in U1 — the ISA rejects DoubleRow
   matmuls at dst base 64, so the placement comes from the stationary
   column index), so one Sin and one output DMA cover both jobs;
 - single-bank (pad<=512) slots drain chunk pairs in one fused op;
 - act-table loads are forced into the PE warm-up window by dummy
   activations, and dot pieces are interleaved between projection chunk
   groups so the sign engines never starve behind the PE's dot blocks.
Output is written fp16 (sin in [-1,1]; exact to 2^-12) and upcast on the
host. Single-tf32 projection puts the end-to-end rel err at ~7e-3
(theory for tf32 operands: ~9e-3), ~3x inside the 2e-2 gate.
"""
import os
import sys

sys.path.insert(0, "/opt/trn_rl_repo")

from contextlib import ExitStack

import numpy as np

import concourse.bass as bass
import concourse.mybir as mybir
import concourse.tile as tile
from concourse import bacc
from concourse.bass_utils import run_bass_kernel_spmd

L, BAT, A, BDOC, D, NB = 2, 32, 64, 1024, 128, 1024
CORES = 8
BPC = BAT // CORES          # batch slots per core
CH = NB // 128              # 8 bit-chunks
SCALE = float(2.0 ** 66)
PI = float(np.pi)

F32 = mybir.dt.float32
F32R = mybir.dt.float32r
BF16 = mybir.dt.bfloat16
FP16 = mybir.dt.float16
FP8 = mybir.dt.float8e4
Alu = mybir.AluOpType
Act = mybir.ActivationFunctionType

NWARM = 7                   # PE ramp dummy matmuls (512 cols each)

# schedule-tuning knobs (values picked by the offline TimelineSim sweep)
TUNE = {
    "nwarm": 6,
    "dve_bias": 550.0,      # initial ns handicap on DVE in the sign balancer
    "interleave": True,     # emit dot pieces between projection chunk groups
    "job_order": "desc",    # slot order: desc | asc | valley
    "jitter": 70,           # balancer tie-break jitter seed (0 = off)
    "ehp_bufs": 4,
    "u2p_bufs": 4,
    "act_tail_dma": False,  # issue the final output DMA from the ACT queue
}

_BUILD_CACHE: dict = {}


def _col_splits(n):
    """Split [0, n) into equal-width pieces of <=512 columns (>=256 keeps
    float32r matmuls at full rate; a matmul may not cross a PSUM bank, so
    piece i is written at PSUM column 512*i). Equal widths mean one strided
    [p, npieces, w] access pattern covers all pieces, so sign/sin run as a
    single instruction per chunk. Returns (c0, c1, p0) per piece."""
    npieces = -(-n // 512)
    w = -(-(n // npieces) // 16) * 16
    while w * npieces < n:
        w += 16
    assert w * npieces >= n and w <= 512 and npieces <= 2
    return [(i * w, min((i + 1) * w, n), 512 * i) for i in range(npieces)]


def _slot_order_of(pads_c):
    desc = sorted(range(len(pads_c)), key=lambda s: -pads_c[s])
    if TUNE["job_order"] == "asc":
        return desc[::-1]
    if TUNE["job_order"] == "valley":
        # big slots in the middle, small first and last
        asc = desc[::-1]
        return [asc[0]] + desc[:-1]
    return desc


def _sign_plan(pads_c, qpad):
    """Assign each (job, chunk) doc sign to 'dve' or 'act' greedily by
    modelled engine cost (ns): DVE tensor_scalar = free*1.042 + 125,
    ACT activation = free*0.833 + 143. Costs are charged at the point in
    the job stream where the work actually runs (sin of job j-1 lands
    during job j; query-pair signs land around job 1; act-table loads are
    hoisted to the idle warm-up window) so the split is balanced in TIME,
    not just in total. Jobs run slots in descending-pad order, L-major."""
    QW = BPC * L * qpad
    order = _slot_order_of(pads_c)
    jobs = [(b, l) for b in order for l in range(L)]
    rng = None
    if TUNE["jitter"]:
        import random
        rng = random.Random(TUNE["jitter"])
    dve = float(TUNE["dve_bias"])
    act = 0.0
    plan = []
    for j, (b, _l) in enumerate(jobs):
        if j == 2:
            dve += 2 * (2 * QW * 1.042 + 125.0)
            act += 2 * (2 * QW * 0.833 + 185.0)  # 2 fused pairs per engine
        if j in (2, 4, 6) and j > 2:
            # fused L-pair Sin epilogues land during jobs 4/6 (first at 2)
            act += pads_c[jobs[j - 4][0]] * 0.833 + 212.0
        if j == 3:
            act += pads_c[jobs[0][0]] * 0.833 + 212.0
        row = []
        jit = (lambda: 1.0 + 0.25 * (rng.random() - 0.5)) if rng else (lambda: 1.0)
        if pads_c[b] <= 512:
            # np=1 slots drain chunk PAIRS in one fused op
            for _kk in range(CH // 2):
                cd = (2 * pads_c[b] * 1.042 + 125.0) * jit()
                ca = (2 * pads_c[b] * 0.833 + 185.0) * jit()
                if dve + cd <= act + ca:
                    dve += cd
                    row += ["dve", None]
                else:
                    act += ca
                    row += ["act", None]
        else:
            for _k in range(CH):
                cd = (pads_c[b] * 1.042 + 125.0) * jit()
                ca = (pads_c[b] * 0.833 + 185.0) * jit()
                if dve + cd <= act + ca:
                    dve += cd
                    row.append("dve")
                else:
                    act += ca
                    row.append("act")
        plan.append(row)
    return plan


def _build(pads_c: tuple, qpad: int = A, reps: int = 1):
    """Per-core SPMD program. pads_c[b]: compute width (mult of 16) of batch
    slot b. reps > 1 re-emits the whole body (timing instrumentation only)."""
    pads_c = tuple(int(p) for p in pads_c)
    pad_cmax = max(pads_c)
    slot_splits = [_col_splits(p) for p in pads_c]
    QW = BPC * L * qpad
    sign_plan = _sign_plan(pads_c, qpad)

    nc = bacc.Bacc("TRN2", target_bir_lowering=False, debug=False)

    QE = nc.dram_tensor("qe", [D, QW], F32R, kind="ExternalInput").ap()
    DE = nc.dram_tensor("de", [BPC, L, D, pad_cmax], F32R, kind="ExternalInput").ap()
    RT = nc.dram_tensor("rt", [D, NB], F32R, kind="ExternalInput").ap()
    OUT = nc.dram_tensor("out", [BPC, L * qpad, pad_cmax], FP16, kind="ExternalOutput").ap()

    def sign_to(eng, u, pp):
        if eng == "dve":
            nc.vector.tensor_scalar(u, pp, 1.0, -1.0, Alu.min, Alu.max)
        else:
            nc.scalar.activation(u, pp, Act.Sign)

    with tile.TileContext(nc) as tc, ExitStack() as ctx:
        const = ctx.enter_context(tc.tile_pool(name="const", bufs=1))
        ehp = ctx.enter_context(tc.tile_pool(name="ehp",
                                             bufs=int(TUNE["ehp_bufs"])))
        u2p = ctx.enter_context(tc.tile_pool(name="u2p",
                                             bufs=int(TUNE["u2p_bufs"])))
        outp = ctx.enter_context(tc.tile_pool(name="outp", bufs=2))
        # 8 PSUM banks: 2 x 2-bank rotating chunk tiles + 2 x 2-bank S tiles
        # (two S bufs so job j's dots never wait on job j-1's Sin drain)
        ps_p = ctx.enter_context(tc.tile_pool(name="ps_p", bufs=3, space="PSUM"))
        ps_s = ctx.enter_context(tc.tile_pool(name="ps_s", bufs=1, space="PSUM"))

        for _rep in range(reps):
            _rp = f"r{_rep}_"
            # ---- constants: rt arrives in pieces so the first projection
            # chunk unblocks as early as possible; everything lands directly
            # in float32r tiles (host pre-rounds to tf32) ----
            rhl = const.tile([D, NB], F32R, tag="rhl", name=f"{_rp}rhl")
            qh = const.tile([D, QW], F32R, tag="qh", name=f"{_rp}qh")
            # U1 block per (chunk k, slot b) is 192 cols: [q(l=0) | 64 zeros
            # | q(l=1)].  The dot's stationary for job (b,l) is the 128-col
            # window at l*64: [q_l0|zeros] or [zeros|q_l1], which places job
            # l's S rows at partitions l*64 of a SINGLE accumulation chain —
            # PE-free L-pair fusion of the epilogue (dst base stays 0; the
            # ISA rejects DoubleRow matmuls with dst base 64).
            UB = 3 * qpad                     # 192: U1 block width
            U1 = const.tile([D, CH * BPC * UB], FP8, tag="U1",
                            name=f"{_rp}U1")
            u1_pitch, u1_off = U1[:].ap[0][0], U1[:].offset
            # zero the hole strips once (Pool is otherwise idle)
            holes = bass.AP(U1.tensor, u1_off + qpad,
                            [[u1_pitch, D], [UB, CH * BPC], [1, qpad]])

            _slot_order = _slot_order_of(pads_c)
            jobs = [(b, l) for b in _slot_order for l in range(L)]
            st = [dict() for _ in jobs]

            def stage_a(j, split=False):
                b, l = jobs[j]
                pad = pads_c[b]
                eh = ehp.tile([D, pad_cmax], F32R, tag="eh",
                              name=f"{_rp}eh{j}")[:, 0:pad]
                if split:
                    # piece-granular DMAs: the first projection piece can
                    # start one transfer earlier (deps are range-tracked)
                    for c0, c1, _ in slot_splits[b]:
                        nc.sync.dma_start(out=eh[:, c0:c1],
                                          in_=DE[b, l, :, c0:c1])
                else:
                    nc.sync.dma_start(out=eh, in_=DE[b, l, :, 0:pad])
                st[j]["eh"] = eh

            # DMA priority order: first doc job, first proj chunk weights,
            # the rest of the weights, second doc job, queries.
            nc.sync.dma_start(out=rhl[:, 0:128], in_=RT[:, 0:128])
            stage_a(0)
            nc.sync.dma_start(out=rhl[:, 128:512], in_=RT[:, 128:512])
            nc.sync.dma_start(out=rhl[:, 512:NB], in_=RT[:, 512:NB])
            stage_a(1)
            nc.sync.dma_start(out=qh, in_=QE)

            # PE pre-warm: dependency-free dummy matmuls pull the PE through
            # its cold/mid clock ramp while the first DMAs land, so the real
            # projections run at 2.4 GHz
            warm = const.tile([D, 512], BF16, tag="warm", name=f"{_rp}warm")
            nc.gpsimd.memset(warm, 0.0)
            # dummy Sign + Sin on the idle ACT engine so both act-table
            # loads are hoisted into the warm-up window instead of stalling
            # the pipeline at their first real use
            wact = const.tile([D, 32], BF16, tag="wact", name=f"{_rp}wact")
            nc.scalar.activation(wact[:, 0:16], warm[:, 0:16], Act.Sign)
            nc.scalar.activation(wact[:, 16:32], warm[:, 16:32], Act.Sin)
            wps = ps_p.tile([D, 1024], F32, tag="pp",
                            name=f"{_rp}wps")[:, 0:512]
            for i in range(int(TUNE["nwarm"])):
                nc.tensor.matmul(wps, warm[:, 0:128], warm,
                                 start=True, stop=True)
            # zero U1's hole strips once the warm tile is set (Pool is
            # otherwise idle; emitting it here keeps the warm memset first
            # in Pool's queue so the PE ramp starts immediately)
            nc.gpsimd.memset(holes, 0.0)

            def stage_b(j, ks):
                b, l = jobs[j]
                pad = pads_c[b]
                splits = slot_splits[b]
                npieces = len(splits)
                w = splits[0][1] - splits[0][0]
                assert npieces * w == pad
                eh = st[j]["eh"]
                if "U2" not in st[j]:
                    st[j]["U2"] = u2p.tile([D, CH * pad_cmax], FP8, tag="U2",
                                           name=f"{_rp}U2{j}")
                U2 = st[j]["U2"]
                if npieces == 1:
                    # single-bank projections: chunk pairs share one 2-bank
                    # tile (cols 0 and 512) so the sign drains both chunks
                    # in one op — halves the per-op PSUM-access bubbles
                    for k0 in [k for k in ks if k % 2 == 0]:
                        pp = ps_p.tile([D, 1024], F32, tag="pp",
                                       name=f"{_rp}pp{j}_{k0}")
                        for h in range(2):
                            k = k0 + h
                            nc.tensor.matmul(pp[:, 512 * h:512 * h + pad],
                                             rhl[:, k * 128:(k + 1) * 128],
                                             eh, start=True, stop=True)
                        ppv = pp[:].rearrange("p (n c) -> p n c",
                                              c=512)[:, 0:2, 0:pad]
                        u2v = U2[:, k0 * pad:(k0 + 2) * pad] \
                            .rearrange("p (n c) -> p n c", c=pad)
                        sign_to(sign_plan[j][k0], u2v, ppv)
                    return
                for k in ks:
                    rh_k = rhl[:, k * 128:(k + 1) * 128]
                    pp = ps_p.tile([D, 1024], F32, tag="pp",
                                   name=f"{_rp}pp{j}_{k}")
                    for c0, c1, p0 in splits:
                        nc.tensor.matmul(pp[:, p0:p0 + c1 - c0], rh_k,
                                         eh[:, c0:c1], start=True, stop=True)
                    ppv = pp[:].rearrange("p (n c) -> p n c",
                                          c=512)[:, 0:npieces, 0:w]
                    u2v = U2[:, k * pad:(k + 1) * pad] \
                        .rearrange("p (n c) -> p n c", c=w)
                    sign_to(sign_plan[j][k], u2v, ppv)

            def query_proj():
                # chunk pairs share one PSUM tile (cols 0 and 512); one
                # 5-level-AP sign per pair scatters both chunks' [b, l, q]
                # into their holey U1 blocks, alternating DVE/ACT so U1
                # completes before the first dot
                for kk in range(CH // 2):
                    qp = ps_p.tile([D, 1024], F32, tag="pp",
                                   name=f"{_rp}qp{kk}")
                    for h in range(2):
                        k = 2 * kk + h
                        nc.tensor.matmul(qp[:, 512 * h:512 * h + QW],
                                         rhl[:, k * 128:(k + 1) * 128], qh,
                                         start=True, stop=True)
                    u1v = bass.AP(U1.tensor, u1_off + 2 * kk * BPC * UB,
                                  [[u1_pitch, D], [BPC * UB, 2], [UB, BPC],
                                   [2 * qpad, L], [1, qpad]])
                    qpv = bass.AP(qp.tensor, qp[:].offset,
                                  [[qp[:].ap[0][0], D], [512, 2],
                                   [L * qpad, BPC], [qpad, L], [1, qpad]])
                    sign_to("dve" if kk % 2 == 0 else "act", u1v, qpv)

            def stage_c_piece(p, pi):
                """Dot chain for piece pi of the fused L-pair p (jobs 2p,
                2p+1 — same slot b, l=0/1): one 8-matmul accumulation chain
                writes both jobs' S rows (job l at partitions [64l, 64l+64)
                via the zero-padded stationary windows)."""
                j0 = 2 * p
                b, _ = jobs[j0]
                pad = pads_c[b]
                if "S" not in st[j0]:
                    st[j0]["S"] = ps_s.tile([L * qpad, 1024], F32, tag="S",
                                            name=f"{_rp}S{p}")
                S = st[j0]["S"]
                CW = BPC * UB
                c0, c1, p0 = slot_splits[b][pi]
                for i, j in enumerate((j0, j0 + 1)):
                    _, l = jobs[j]
                    U2 = st[j]["U2"]
                    for jj in range(CH // 2):
                        lw = U1[:, 2 * jj * CW:(2 * jj + 2) * CW] \
                            .rearrange("p (o c) -> p o c", o=2) \
                            [:, :, b * UB + l * qpad:b * UB + (l + 2) * qpad]
                        rv = U2[:, 2 * jj * pad:(2 * jj + 2) * pad] \
                            .rearrange("p (o c) -> p o c", o=2)[:, :, c0:c1]
                        nc.tensor.matmul(
                            S[:, p0:p0 + c1 - c0], lw, rv,
                            start=(i == 0 and jj == 0),
                            stop=(i == 1 and jj == CH // 2 - 1),
                            perf_mode=mybir.MatmulPerfMode.DoubleRow,
                        )

            def stage_c_tail(p):
                """Sin + output DMA for fused pair p (after all its dot
                pieces): a single Sin over all 128 partitions and a single
                DMA (the L dim is contiguous in OUT)."""
                j0 = 2 * p
                b, _ = jobs[j0]
                pad = pads_c[b]
                splits = slot_splits[b]
                npieces = len(splits)
                w = splits[0][1] - splits[0][0]
                S = st[j0]["S"]
                sim = outp.tile([L * qpad, pad_cmax], FP16, tag="sim",
                                name=f"{_rp}sim{p}")[:, 0:pad]
                if npieces == 1:
                    nc.scalar.activation(sim, S[:, 0:pad], Act.Sin,
                                         scale=PI / (2.0 * NB))
                else:
                    sv = S[:].rearrange("p (n c) -> p n c",
                                        c=512)[:, 0:npieces, 0:w]
                    mv = sim.rearrange("p (n c) -> p n c", c=w)
                    nc.scalar.activation(mv, sv, Act.Sin, scale=PI / (2.0 * NB))
                if TUNE["act_tail_dma"] and p == len(jobs) // 2 - 1:
                    # same-engine issue skips the ACT->SP semaphore hop on
                    # the drain-critical final output
                    nc.scalar.dma_start(out=OUT[b, :, 0:pad], in_=sim)
                else:
                    nc.sync.dma_start(out=OUT[b, :, 0:pad], in_=sim)

            # deeper pipeline: the fused epilogue of pair p is emitted in
            # the middle of b(2p+2), so dots/Sin/output overlap the next
            # pair's projections and the Sin lands mid-stream on ACT
            # instead of stalling it
            n = len(jobs)
            stage_b(0, range(CH))
            stage_a(2)
            stage_b(1, range(CH))
            query_proj()
            stage_a(3)
            for j in range(2, n):
                ci = j // 2 - 1 if j % 2 == 0 else None   # fused pair idx
                cn = len(slot_splits[jobs[2 * ci][0]]) if ci is not None else 0
                # np=1 slots sign chunk PAIRS, so their groups stay aligned
                if TUNE["interleave"]:
                    g = ((0, 2), (2, 4), (4, CH)) \
                        if len(slot_splits[jobs[j][0]]) == 1 \
                        else ((0, 3), (3, 5), (5, CH))
                    stage_b(j, range(*g[0]))
                    if ci is not None:
                        stage_c_piece(ci, 0)
                    stage_b(j, range(*g[1]))
                    if ci is not None:
                        if cn > 1:
                            stage_c_piece(ci, 1)
                        stage_c_tail(ci)
                    if j + 2 < n:
                        stage_a(j + 2)
                    stage_b(j, range(*g[2]))
                else:
                    stage_b(j, range(0, 4))
                    if ci is not None:
                        stage_c_piece(ci, 0)
                        if cn > 1:
                            stage_c_piece(ci, 1)
                        stage_c_tail(ci)
                    if j + 2 < n:
                        stage_a(j + 2)
                    stage_b(j, range(4, CH))
            stage_c_piece(n // 2 - 1, 0)
            if len(slot_splits[jobs[n - 2][0]]) > 1:
                stage_c_piece(n // 2 - 1, 1)
            stage_c_tail(n // 2 - 1)

    nc.compile()
    return nc


def _tf32(x):
    """Round-to-nearest-even fp32 -> tf32 (11-bit mantissa), bit-matching
    the PE's fp32_to_fp32r conversion."""
    u = np.ascontiguousarray(x, np.float32).view(np.uint32).astype(np.uint64)
    u = (u + 0x07FF + ((u >> 12) & 1)) & 0xFFFFFFFFFFFFF000
    return (u & 0xFFFFFFFF).astype(np.uint32).view(np.float32)


def _stage_inputs(query_embed, doc_embed, query_tok, doc_tok, r):
    query_embed = np.ascontiguousarray(query_embed, dtype=np.float32)
    doc_embed = np.ascontiguousarray(doc_embed, dtype=np.float32)
    r = np.ascontiguousarray(r, dtype=np.float32)

    qmask = (np.asarray(query_tok) != 0)
    dmask = (np.asarray(doc_tok) != 0)

    # sort batches by active count; slot s takes ranks [s*CORES, (s+1)*CORES)
    # spread across the 8 cores, so per-slot padding is tight and identical
    # on every core (SPMD requires one shape per slot)
    counts = dmask.sum(axis=1).astype(int)
    order = np.argsort(counts, kind="stable")
    assign = np.empty((CORES, BPC), dtype=int)   # assign[c, b] = batch id
    for s in range(BPC):
        for c in range(CORES):
            assign[c, s] = order[s * CORES + c]
    def _pad(n):
        # mult of 16; slots that split across two PSUM banks need halves
        # that are themselves mult of 16, so round those to mult of 32
        p = max(64, -(-n // 16) * 16)
        if p > 512:
            p = -(-n // 32) * 32
        return min(BDOC, p)

    pads_c = tuple(_pad(int(counts[assign[:, s]].max())) for s in range(BPC))
    pad_cmax = max(pads_c)

    # all 64 query rows are kept (masked rows zeroed -> zero codes -> zero
    # output rows, matching the reference); 64 keeps the two jobs of an
    # L-pair at PE-legal PSUM base partitions 0 and 64
    qe_m = query_embed * qmask[None, :, :, None].astype(np.float32)
    qpad = A
    rt = np.ascontiguousarray(_tf32(r.T * SCALE))

    idxs = [np.flatnonzero(dmask[g]) for g in range(BAT)]
    in_maps = []
    for c in range(CORES):
        # embeddings staged pre-transposed [D, tokens], pre-rounded to tf32
        # (value-exact under the f32r DMA interpretation)
        qe_c = np.zeros((D, BPC * L * qpad), dtype=np.float32)
        de_c = np.zeros((BPC, L, D, pad_cmax), dtype=np.float32)
        for b in range(BPC):
            g = assign[c, b]
            for li in range(L):
                col = (b * L + li) * qpad
                qe_c[:, col:col + A] = qe_m[li, g].T
            idx = idxs[g]
            de_c[b, :, :, :len(idx)] = doc_embed[:, g, idx].transpose(0, 2, 1)
        in_maps.append({"qe": _tf32(qe_c), "de": _tf32(de_c), "rt": rt})

    return in_maps, assign, idxs, pads_c, qpad


def kernel(query_embed, doc_embed, query_tok, doc_tok, r):
    in_maps, assign, idxs, pads_c, qpad = _stage_inputs(
        query_embed, doc_embed, query_tok, doc_tok, r)

    key = (pads_c, qpad)
    if key not in _BUILD_CACHE:
        _BUILD_CACHE[key] = _build(pads_c, qpad)
    nc = _BUILD_CACHE[key]

    res = run_bass_kernel_spmd(nc, in_maps, core_ids=list(range(CORES)))

    out = np.zeros((BAT, L, A, BDOC), dtype=np.float32)
    for c in range(CORES):
        o_c = np.asarray(res.results[c]["out"]).astype(np.float32)
        o_c = o_c.reshape(BPC, L, A, -1)
        for b in range(BPC):
            g = assign[c, b]
            idx = idxs[g]
            for li in range(L):
                out[g, li][:, idx] = o_c[b, li, :, :len(idx)]
    return out
